# revision 1
# baseline (speedup 1.0000x reference)
"""GroupDRO segment-reduce kernel for 8 Trainium2 NeuronCores.

Algorithm:
  - Shard the 2^24 elements across 8 cores (2M each), laid out [128, 16384].
  - Per core, per 128-element chunk (one SBUF column), decompose each
    group id g = hi*128 + lo (hi in [0,79), lo in [0,128)). Build a bf16
    one-hot of lo (stationary [128el, 128lo]) and a moving matrix
    [128el, 160] = [mask_hi * loss | mask_hi]; a single PE matmul per chunk
    accumulates sums[lo, hi] and counts[lo, hi] into one PSUM tile.
  - AllReduce the (128 x 160) partial sums/counts over the 8 cores.
  - Finalize on-device: mean = sums/max(counts,1); w = gw*exp(0.01*mean);
    out = sum(w*mean)/sum(w).
"""

import os
import sys

import numpy as np

sys.path.insert(0, "/opt/trn_rl_repo")

import concourse.bacc as bacc
import concourse.tile as tile
from concourse import mybir
from concourse.bass_utils import run_bass_kernel_spmd

NUM_GROUPS = 10000
STEP_SIZE = 0.01
B = 16777216
NCORES = 8
P = 128
LO_W = 128
HI_W = 80  # hi in [0, 79); padded to 80 (even) for alignment
E_PER_CORE = B // NCORES          # 2,097,152
FREE = E_PER_CORE // P            # 16384 columns per core
FT = 512                          # columns per DMA tile
FC = 64                           # columns per build sub-batch

F32 = mybir.dt.float32
I32 = mybir.dt.int32
I16 = mybir.dt.int16
BF16 = mybir.dt.bfloat16


def _ecopy(eng, out, in_):
    if hasattr(eng, "tensor_copy"):
        eng.tensor_copy(out=out, in_=in_)
    else:
        eng.copy(out=out, in_=in_)


def _double_fill(eng, t, fc, width):
    """Fill t[:, :fc, 1:width] by doubling copies of t[:, :fc, 0:...]."""
    filled = 1
    while filled < width:
        n = min(filled, width - filled)
        _ecopy(eng, t[:, 0:fc, filled:filled + n], t[:, 0:fc, 0:n])
        filled += n


def _affine_double_fill(nc, eng, t, fc, width):
    """Fill t so that t[:, f, j] = t[:, f, 0] - j, by doubling copies that
    subtract the offset (DVE tensor_scalar at 4x, or ACT activation bias)."""
    filled = 1
    while filled < width:
        n = min(filled, width - filled)
        dst = t[:, 0:fc, filled:filled + n]
        src = t[:, 0:fc, 0:n]
        if eng is nc.vector:
            eng.tensor_scalar(out=dst, in0=src, scalar1=float(-filled),
                              scalar2=None, op0=mybir.AluOpType.add)
        else:
            eng.activation(out=dst, in_=src,
                           func=mybir.ActivationFunctionType.Copy,
                           bias=float(-filled), scale=1.0)
        filled += n


def _build_program(free=FREE, ft=FT, fc=FC, ncores=NCORES, debug=False):
    nc = bacc.Bacc("TRN2", target_bir_lowering=False, debug=debug,
                   num_devices=ncores)

    losses_d = nc.dram_tensor("losses", [P, free], F32, kind="ExternalInput")
    gids_d = nc.dram_tensor("gids", [P, free], I32, kind="ExternalInput")
    gw_d = nc.dram_tensor("gw", [P, HI_W], F32, kind="ExternalInput")
    out_d = nc.dram_tensor("out", [1, 1], F32, kind="ExternalOutput")

    n_tiles = free // ft
    n_sub = ft // fc

    with tile.TileContext(nc) as tc:
        with (
            tc.tile_pool(name="const", bufs=1) as cpool,
            tc.tile_pool(name="inp", bufs=2) as ipool,
            tc.tile_pool(name="dig", bufs=2) as dpool,
            tc.tile_pool(name="build",
                         bufs=int(os.environ.get("K_BUFS", 3))) as bpool,
            tc.tile_pool(name="fin", bufs=1) as fpool,
            tc.tile_pool(name="psum", bufs=1, space="PSUM") as ppool,
            tc.tile_pool(name="dram", bufs=1, space="DRAM") as drpool,
        ):
            psum_acc = ppool.tile([P, 2 * HI_W], F32, space="PSUM")

            first = True
            for t in range(n_tiles):
                sl = slice(t * ft, (t + 1) * ft)
                L32 = ipool.tile([P, ft], F32, tag="L32")
                G32 = ipool.tile([P, ft], I32, tag="G32")
                nc.sync.dma_start(out=L32[:], in_=losses_d.ap()[:, sl])
                nc.sync.dma_start(out=G32[:], in_=gids_d.ap()[:, sl])

                hi32 = dpool.tile([P, ft], I32, tag="hi32")
                lo32 = dpool.tile([P, ft], I32, tag="lo32")
                nc.vector.tensor_scalar(out=hi32[:], in0=G32[:], scalar1=7,
                                        scalar2=None,
                                        op0=mybir.AluOpType.logical_shift_right)
                nc.vector.tensor_scalar(out=lo32[:], in0=G32[:], scalar1=127,
                                        scalar2=None,
                                        op0=mybir.AluOpType.bitwise_and)
                lo_bf = dpool.tile([P, ft], BF16, tag="lo_bf")
                hi_bf = dpool.tile([P, ft], BF16, tag="hi_bf")
                L_bf = dpool.tile([P, ft], BF16, tag="L_bf")
                nc.vector.tensor_copy(out=lo_bf[:], in_=lo32[:])
                nc.vector.tensor_copy(out=hi_bf[:], in_=hi32[:])
                nc.scalar.copy(out=L_bf[:], in_=L32[:])

                lo_dw = int(os.environ.get("K_LO_DW", 32))
                hi_dw = int(os.environ.get("K_HI_DW", 20))
                for s in range(n_sub):
                    ss = slice(s * fc, (s + 1) * fc)
                    lo_D = bpool.tile([P, fc, lo_dw], BF16, tag="lo_D")
                    oh_lo = bpool.tile([P, fc, LO_W], BF16, tag="oh_lo")
                    hi_D = bpool.tile([P, fc, hi_dw], BF16, tag="hi_D")
                    L_rep = bpool.tile([P, fc, hi_dw], BF16, tag="L_rep")
                    mov = bpool.tile([P, fc, 2, HI_W], BF16, tag="mov")

                    # lo_D[:, f, j] = lo - j   (DVE, 4x tensor_scalar)
                    nc.vector.tensor_copy(out=lo_D[:, :, 0:1], in_=lo_bf[:, ss])
                    _affine_double_fill(nc, nc.vector, lo_D, fc, lo_dw)
                    # hi_D[:, f, j] = hi - j   (ACT, activation bias)
                    nc.scalar.copy(out=hi_D[:, :, 0:1], in_=hi_bf[:, ss])
                    _affine_double_fill(nc, nc.scalar, hi_D, fc, hi_dw)
                    # L_rep[:, f, j] = loss    (ACT, plain doublings)
                    nc.scalar.copy(out=L_rep[:, :, 0:1], in_=L_bf[:, ss])
                    _double_fill(nc.scalar, L_rep, fc, hi_dw)

                    # one-hots: compare-to-offset (DVE single-src, 4x)
                    for off in range(0, LO_W, lo_dw):
                        nc.vector.tensor_scalar(
                            out=oh_lo[:, :, off:off + lo_dw], in0=lo_D[:],
                            scalar1=float(off), scalar2=None,
                            op0=mybir.AluOpType.is_equal)
                    for off in range(0, HI_W, hi_dw):
                        nc.vector.tensor_scalar(
                            out=mov[:, :, 1, off:off + hi_dw], in0=hi_D[:],
                            scalar1=float(off), scalar2=None,
                            op0=mybir.AluOpType.is_equal)
                        # spread = (hi_D == off) * loss  (DVE fused)
                        nc.vector.scalar_tensor_tensor(
                            out=mov[:, :, 0, off:off + hi_dw],
                            in0=hi_D[:], scalar=float(off),
                            in1=L_rep[:],
                            op0=mybir.AluOpType.is_equal,
                            op1=mybir.AluOpType.mult)

                    mm_m = int(os.environ.get("K_MM_M", LO_W))
                    mm_n = int(os.environ.get("K_MM_N", 2 * HI_W))
                    for f in range(fc):
                        is_last = (t == n_tiles - 1 and s == n_sub - 1
                                   and f == fc - 1)
                        nc.tensor.matmul(
                            out=psum_acc[0:mm_m, 0:mm_n],
                            lhsT=oh_lo[:, f, 0:mm_m],
                            rhs=(mov[:, f, :, :] if mm_n == 2 * HI_W
                                 else mov[:, f, 0, 0:mm_n]),
                            start=first,
                            stop=is_last,
                        )
                        first = False

            # ---- cross-core AllReduce of [P, 160] partials
            acc_sb = fpool.tile([P, 2 * HI_W], F32)
            nc.vector.tensor_copy(out=acc_sb[:], in_=psum_acc[:])
            cc_in = drpool.tile([P, 2 * HI_W], F32)
            cc_out = drpool.tile([P, 2 * HI_W], F32)
            nc.sync.dma_start(out=cc_in[:], in_=acc_sb[:])
            if ncores > 1:
                nc.gpsimd.collective_compute(
                    "AllReduce",
                    mybir.AluOpType.add,
                    replica_groups=[list(range(ncores))],
                    ins=[cc_in.opt()],
                    outs=[cc_out.opt()],
                )
            else:
                nc.sync.dma_start(out=cc_out[:], in_=cc_in[:])
            red = fpool.tile([P, 2 * HI_W], F32)
            nc.sync.dma_start(out=red[:], in_=cc_out[:])

            # ---- finalize
            gw_sb = fpool.tile([P, HI_W], F32)
            nc.sync.dma_start(out=gw_sb[:], in_=gw_d.ap())

            sums = red[:, 0:HI_W]
            cnts = red[:, HI_W:2 * HI_W]
            cnt1 = fpool.tile([P, HI_W], F32)
            nc.vector.tensor_scalar_max(out=cnt1[:], in0=cnts, scalar1=1.0)
            rcp = fpool.tile([P, HI_W], F32)
            nc.vector.reciprocal(out=rcp[:], in_=cnt1[:])
            mean = fpool.tile([P, HI_W], F32)
            nc.vector.tensor_tensor(out=mean[:], in0=sums, in1=rcp[:],
                                    op=mybir.AluOpType.mult)
            ew = fpool.tile([P, HI_W], F32)
            nc.scalar.activation(out=ew[:], in_=mean[:],
                                 func=mybir.ActivationFunctionType.Exp,
                                 scale=STEP_SIZE)
            w = fpool.tile([P, HI_W], F32)
            nc.vector.tensor_tensor(out=w[:], in0=ew[:], in1=gw_sb[:],
                                    op=mybir.AluOpType.mult)
            wm = fpool.tile([P, HI_W], F32)
            nc.vector.tensor_tensor(out=wm[:], in0=w[:], in1=mean[:],
                                    op=mybir.AluOpType.mult)
            pair = fpool.tile([P, 2], F32)
            nc.vector.tensor_reduce(out=pair[:, 0:1], in_=w[:],
                                    axis=mybir.AxisListType.X,
                                    op=mybir.AluOpType.add)
            nc.vector.tensor_reduce(out=pair[:, 1:2], in_=wm[:],
                                    axis=mybir.AxisListType.X,
                                    op=mybir.AluOpType.add)
            ones = fpool.tile([P, 1], F32)
            nc.vector.memset(ones[:], 1.0)
            psum_fin = ppool.tile([1, 2], F32, space="PSUM", tag="psum_fin")
            nc.tensor.matmul(out=psum_fin[:], lhsT=ones[:], rhs=pair[:],
                             start=True, stop=True)
            fin = fpool.tile([1, 2], F32)
            nc.vector.tensor_copy(out=fin[:], in_=psum_fin[:])
            den_r = fpool.tile([1, 1], F32)
            nc.vector.reciprocal(out=den_r[:], in_=fin[:, 0:1])
            res = fpool.tile([1, 1], F32)
            nc.vector.tensor_tensor(out=res[:], in0=fin[:, 1:2], in1=den_r[:],
                                    op=mybir.AluOpType.mult)
            nc.sync.dma_start(out=out_d.ap(), in_=res[:])

    nc.compile()
    return nc


def _double_fill_outer(eng, t, fc, width):
    """Fill t[:, 1:fc, :] by doubling copies of t[:, 0:n, :]."""
    filled = 1
    while filled < fc:
        n = min(filled, fc - filled)
        _ecopy(eng, t[:, filled:filled + n, 0:width], t[:, 0:n, 0:width])
        filled += n


_NC_CACHE = {}


def _get_program(free=FREE, ft=FT, fc=FC):
    key = (free, ft, fc)
    if key not in _NC_CACHE:
        _NC_CACHE[key] = _build_program(free, ft, fc)
    return _NC_CACHE[key]


def _prep_inputs(losses, group_ids, group_weights, free=FREE):
    losses = np.asarray(losses, dtype=np.float32)
    group_ids = np.asarray(group_ids, dtype=np.int32)
    gw = np.asarray(group_weights, dtype=np.float32)
    n = NCORES * P * free
    l_sh = losses[:n].reshape(NCORES, P, free)
    g_sh = group_ids[:n].reshape(NCORES, P, free)
    gw_grid = np.zeros(P * HI_W, dtype=np.float32)
    gw_grid[:NUM_GROUPS] = gw
    gw_grid = np.ascontiguousarray(gw_grid.reshape(HI_W, P).T)
    in_maps = [
        {"losses": np.ascontiguousarray(l_sh[i]),
         "gids": np.ascontiguousarray(g_sh[i]),
         "gw": gw_grid}
        for i in range(NCORES)
    ]
    return in_maps


def kernel(losses, group_ids, group_weights, **run_kwargs):
    nc = _get_program()
    in_maps = _prep_inputs(losses, group_ids, group_weights)
    res = run_bass_kernel_spmd(nc, in_maps, list(range(NCORES)), **run_kwargs)
    out = np.float32(res.results[0]["out"][0, 0])
    kernel.last_results = res
    return np.array(out, dtype=np.float32)


if __name__ == "__main__":
    rng = np.random.default_rng(0)
    losses = rng.random(B, dtype=np.float32)
    gids = rng.integers(0, NUM_GROUPS, B, dtype=np.int32)
    gw = np.ones(NUM_GROUPS, dtype=np.float32) / NUM_GROUPS
    got = kernel(losses, gids, gw)
    # numpy reference
    sums = np.bincount(gids, weights=losses, minlength=NUM_GROUPS)
    cnts = np.bincount(gids, minlength=NUM_GROUPS)
    gl = np.where(cnts > 0, sums / np.maximum(cnts, 1), 0.0)
    w = gw * np.exp(STEP_SIZE * gl)
    w = w / w.sum()
    exp = float((w * gl).sum())
    print("got", got, "exp", exp, "rel", abs(got - exp) / abs(exp))

